# revision 26
# baseline (speedup 1.0000x reference)
"""Trainium2 Bass kernel for the sparse-attention (local 3x3 unfold) problem.

Math (per batch-channel (b,c), H=W=128, K=3, pad=1):
  ku = unfold(key)  -> [9, L] raw-flat, reinterpreted [L, 9]
  qu = unfold(query)
  out1 = ku * qu[:, 4:5] ; out2 = ku[:, 4:5] * qu   (as [L, 9] views)

The flat per-channel output index n in [0, 9L) decomposes two ways:
  * n = 128*q + j           (chunk q = one (patch p2=q//128, row i2=q%128)
                             slice: 128 contiguous floats of a dj-shifted,
                             row-padded image variant)
  * n = 9*g + e             (group g shares one stride-9 "center" factor)

Device layout: channel ch of a group owns 8 partitions (r = 8*ch + rr)
with CHF = 18432 = 144 chunks per partition (n = 18432*rr + f), streamed
as 8 f-sub-tiles of 18 chunks; k/q (and o1/o2) fused on a leading [2]
axis of one DRAM tensor and one SBUF tile.
  * Sub-tile width (2304) % 9 == 0 keeps the stride-9 center-broadcast
    multiply phase-free on every partition (one DVE op per output plane
    covers all 16 channels of a tile).
  * Loads: maximal q-runs constant in (partition, sub-tile, patch) are
    each ONE contiguous DRAM run of a variant image.  One dma_start per
    run covers 16 channels x {k,q} = 32 mutually non-contiguous
    descriptors: the DGE deals descriptors to the 16 SDMA-engine slots
    round-robin from slot 0 and re-aggregates consecutive contiguous
    descriptors, so <16-descriptor (or contiguous-pair) shapes leave
    engines 8-15 idle (HW-measured 86%/39% busy split).
  * Stores: per-channel DRAM is contiguous with offset r*CHF uniform in
    the partition index -> one dma_start per output plane per sub-tile
    (128 descriptors x 4.6 KiB).
  * fp16 end-to-end on device (harness tolerance 2e-2 vs ~7e-4 measured);
    host upcasts to fp32.  Halves HBM read and write traffic vs fp32.

HW-measured dead ends kept out of the design: SBUF->SBUF duplication of
the 3x row overlap between di-patches (engines stream ~20 GB/s
regardless of HBM vs SBUF target, and the extra dependency stage costs
concurrency); dense-center expansion on ACT/GpSimd to unlock the DVE 2x
perf mode (DVE halves but the copies cost more than it saves).

Sharding: pure data-parallel over the 256 (b,c) channels; 32 per core.
"""

import sys

for _p in ("/opt/trn_rl_repo", "/opt/pypackages"):
    if _p not in sys.path:
        sys.path.insert(0, _p)

import numpy as np

import concourse.bass as bass
import concourse.mybir as mybir
import concourse.tile as tile
from concourse.bass import AP
from concourse.bass_utils import run_bass_kernel_spmd
from concourse.vector_clock import ScopedClock

# ---------------------------------------------------------------------------
# Patch: this container's walrus rejects >1 sync-wait on the Tile tail Drain
# ("Too many sync wait commands").  Spill extra waits onto SP NOPs, which
# execute in program order before the all-engine barrier, preserving the
# "all work done before sem clear" semantics.
# ---------------------------------------------------------------------------


def _drain_and_barrier(self, tick_clock, wait_clock):
    nc = self.nc
    drain_inst = nc.sync.drain()
    wait_clock.add_sem_waits(
        drain_inst.ins, ScopedClock({None: tick_clock.global_clock})
    )
    si = drain_inst.ins.sync_info
    if si is not None and len(si.on_wait) > 1:
        waits = list(si.on_wait)
        drain_inst.ins.sync_info = mybir.SyncInfo(
            on_wait=waits[:1], on_update=list(si.on_update)
        )
        for w in waits[1:]:
            nop = nc.sync.nop(nofuse=True)
            nop.ins.sync_info = mybir.SyncInfo(on_wait=[w], on_update=[])

    nc.all_engine_barrier()
    assert self.sems is not None
    popped = nc._tile_sem_poison_stack.pop()
    assert popped is self._sem_poison
    nc.clear_and_free_semaphores(list(self.sems.allocated().values()))
    nc.all_engine_barrier()


tile.TileContext._drain_and_barrier = _drain_and_barrier


def _split_waits(nc, maxw=1):
    """Walrus here allows only `maxw` sync-waits per instruction: move extra
    waits onto same-engine NOPs inserted immediately before the instruction
    (same engine stream => executes before it)."""
    for fn in nc.m.functions:
        for bb in fn.blocks:
            out = []
            for inst in bb.instructions:
                si = getattr(inst, "sync_info", None)
                if si is not None and len(si.on_wait) > maxw:
                    waits = list(si.on_wait)
                    for w in waits[:-maxw]:
                        nop = mybir.InstNoOp(
                            name=nc.get_next_instruction_name(),
                            bass_nofuse=True,
                        )
                        nop.engine = inst.engine
                        nop.sync_info = mybir.SyncInfo(on_wait=[w], on_update=[])
                        nc.register_instruction(nop)
                        out.append(nop)
                    inst.sync_info = mybir.SyncInfo(
                        on_wait=waits[-maxw:], on_update=list(si.on_update)
                    )
                out.append(inst)
            bb.instructions[:] = out

# ---------------------------------------------------------------------------

F16 = mybir.dt.float16

N_CORES = 8
B, C, H, W = 4, 64, 128, 128
BC = B * C                # 256 channels
CPC = BC // N_CORES       # 32 channels per core
NCH = 16                  # channels per tile (x8 partitions = 128)
NG = CPC // NCH           # channel groups per core
HP = H + 2                # padded rows
VAR = HP * W              # one dj-variant: [130, 128]
IMG = 3 * VAR             # three dj-variants per channel
L = H * W
PPCH = 8                  # partitions per channel
CHF = 18432               # elements per partition per channel (144 chunks)
OUT_CH = 9 * L            # 147456 = PPCH * CHF

# Per-group sub-tile chunk widths (must each be a multiple of 9 so the
# stride-9 multiply stays phase-free, and sum to 144).  Tapered: small
# sub-tiles at the pipeline head (first mul starts sooner) and tail
# (short drain after the last loads).
SCHED = [
    [18, 18, 18, 18, 18, 18, 18, 18],
    [18, 18, 18, 18, 18, 18, 18, 18],
]
assert all(sum(s) == 144 and all(w * 128 % 9 == 0 for w in s) for s in SCHED)
MAXW = max(w for s in SCHED for w in s)
MAXF = MAXW * 128         # largest sub-tile free width per fused half


def _segments(sched):
    """Per sub-tile: (FREE, f_base, [(rr, f_off, len, src_off), ...]) --
    maximal q-runs inside the sub-tile window not crossing partition
    (144) or patch (128) boundaries.  f_base = chunk offset within CHF."""
    starts = [0]
    for w in sched:
        starts.append(starts[-1] + w)
    tiles = []
    for t, w in enumerate(sched):
        s0 = starts[t]
        bounds = sorted(
            {q for q in range(0, 1153, 144)}
            | {q for q in range(0, 1153, 128)}
            | {144 * r + s for r in range(8) for s in (s0, s0 + w)}
        )
        segs = []
        for qs, qe in zip(bounds[:-1], bounds[1:]):
            rr = qs // 144
            s = qs - 144 * rr
            if not (s0 <= s < s0 + w):
                continue
            p2 = qs // 128
            di, dj = divmod(p2, 3)
            segs.append(
                (rr, (s - s0) * 128, (qe - qs) * 128,
                 dj * VAR + (qs - 128 * p2 + di) * W)
            )
        tiles.append((w * 128, s0 * 128, segs))
    return tiles


_TILES = [_segments(s) for s in SCHED]


def _build_program():
    nc = bass.Bass(trn_type="TRN2")
    # k and q fused on a leading [2] axis (and o1/o2 likewise) so one
    # dma_start covers both: 32-descriptor loads / 256-descriptor stores
    # halve the dma_start count and keep all 16 SDMA engines 2 deep.
    kq = nc.dram_tensor("kq", [2, CPC, 3, HP, W], F16, kind="ExternalInput")
    oo = nc.dram_tensor("oo", [2, CPC, OUT_CH], F16, kind="ExternalOutput")
    IN_X = CPC * IMG          # DRAM stride between k and q planes
    OUT_X = CPC * OUT_CH      # DRAM stride between o1 and o2 planes

    # Three dynamic DMA queues (SP-HWDGE, ACT-HWDGE, Pool-SWDGE); strict
    # round-robin keeps every queue fed (prior HW finding: greedy
    # bin-packing clusters DMAs per queue and the per-engine FIFO then
    # serializes them).
    engines = [nc.sync, nc.scalar, nc.gpsimd]
    eng_i = [0]

    def eng():
        e = engines[eng_i[0] % len(engines)]
        eng_i[0] += 1
        return e

    MF2 = 2 * MAXF  # allocated fused tile free width: k then q (o1 then o2)

    def do_loads(g, free, segs, ti):
        # 32 descriptors (channel-major x {k,q}) per dma_start, mutually
        # non-contiguous in stream order.  Descriptors are dealt to
        # SDMA-engine slots round-robin from slot 0 and consecutive
        # contiguous descriptors re-aggregate into one packet, so
        # 8-descriptor loads pile onto engines 0-7 (HW-measured: 86%
        # busy vs 39% on engines 8-15); 32 descriptors keep all 16
        # engines 2 deep.
        th = ti[:].tensor
        for rr, f_off, seg_len, src_off in segs:
            eng().dma_start(
                AP(th, rr * MF2 + f_off,
                   [[PPCH * MF2, NCH], [free, 2], [1, seg_len]]),
                AP(kq, g * NCH * IMG + src_off,
                   [[IMG, NCH], [IN_X, 2], [1, seg_len]]),
            )

    def do_mul_store(g, free, f_base, ti, ot):
        ith, oth = ti[:].tensor, ot[:].tensor
        ap_o = [[MF2, 128], [9, free // 9], [1, 9]]
        ap_b = [[MF2, 128], [9, free // 9], [0, 9]]
        # o1 = k_full * q_center ; o2 = q_full * k_center.  Each output's
        # store issues right after its own multiply so the second half of
        # the store traffic isn't gated on both muls (shorter drain).
        # (Dense-center rewrites that unlock the DVE 2x perf mode were
        # HW-measured net-neutral: the ACT/GpSimd expansion copies cost
        # more than the DVE multiply saves, and DMA stays the bottleneck.)
        # DRAM per channel is contiguous: partition r = 8*ch + rr maps
        # to offset r*CHF + f_base, uniform across all 128 partitions.
        for x in (0, 1):
            nc.vector.tensor_mul(
                AP(oth, x * free, ap_o),
                AP(ith, x * free, ap_o),
                AP(ith, (1 - x) * free + 4, ap_b),
            )
            eng().dma_start(
                AP(oo, x * OUT_X + g * NCH * OUT_CH + f_base,
                   [[CHF, 128], [1, free]]),
                AP(oth, x * free, [[MF2, 128], [1, free]]),
            )

    with tile.TileContext(nc) as tc:
        with (
            tc.tile_pool(name="tin", bufs=3) as tin,
            tc.tile_pool(name="tout", bufs=3) as tout,
        ):
            # Software pipeline with one-tile lookahead so loads of tile
            # n+1 sit AHEAD of (mul-gated) stores of tile n in each DMA
            # engine's FIFO -> no head-of-line blocking on the loads.
            prev = None
            for g in range(NG):
                for free, f_base, segs in _TILES[g]:
                    ti = tin.tile([128, MF2], F16, tag="ti")
                    do_loads(g, free, segs, ti)
                    if prev is not None:
                        do_mul_store(*prev)
                    ot = tout.tile([128, MF2], F16, tag="ot")
                    prev = (g, free, f_base, ti, ot)
            do_mul_store(*prev)
    _split_waits(nc)
    return nc


_NC_CACHE = []


def _get_nc():
    if not _NC_CACHE:
        _NC_CACHE.append(_build_program())
    return _NC_CACHE[0]


def _variants(x):
    """[B,C,H,W] -> [BC, 3, HP, W] fp16: dj-shifted, row-padded column
    windows of the zero-padded image."""
    xpad = np.pad(
        np.ascontiguousarray(x, dtype=np.float32).reshape(BC, H, W),
        ((0, 0), (1, 1), (1, 1)),
    )
    v = np.stack([xpad[:, :, j : j + W] for j in range(3)], axis=1)
    return np.ascontiguousarray(v.astype(np.float16))


def make_in_maps(key_map, query_map):
    kv = _variants(key_map)
    qv = _variants(query_map)
    return [
        {
            "kq": np.ascontiguousarray(
                np.stack([kv[m * CPC : (m + 1) * CPC],
                          qv[m * CPC : (m + 1) * CPC]])
            ),
        }
        for m in range(N_CORES)
    ]


def assemble(results):
    out1 = np.concatenate([results[m]["oo"][0] for m in range(N_CORES)], axis=0)
    out2 = np.concatenate([results[m]["oo"][1] for m in range(N_CORES)], axis=0)
    return (
        out1.reshape(B, C, L, 9).astype(np.float32),
        out2.reshape(B, C, L, 9).astype(np.float32),
    )


def kernel(key_map, query_map):
    nc = _get_nc()
    in_maps = make_in_maps(key_map, query_map)
    res = run_bass_kernel_spmd(nc, in_maps, core_ids=list(range(N_CORES)))
    return assemble(res.results)


# revision 27
# speedup vs baseline: 1.0262x; 1.0262x over previous
"""Trainium2 Bass kernel for the sparse-attention (local 3x3 unfold) problem.

Math (per batch-channel (b,c), H=W=128, K=3, pad=1):
  ku = unfold(key)  -> [9, L] raw-flat, reinterpreted [L, 9]
  qu = unfold(query)
  out1 = ku * qu[:, 4:5] ; out2 = ku[:, 4:5] * qu   (as [L, 9] views)

The flat per-channel output index n in [0, 9L) decomposes two ways:
  * n = 128*q + j           (chunk q = one (patch p2=q//128, row i2=q%128)
                             slice: 128 contiguous floats of a dj-shifted,
                             row-padded image variant)
  * n = 9*g + e             (group g shares one stride-9 "center" factor)

Device layout: channel ch of a group owns 8 partitions (r = 8*ch + rr)
with CHF = 18432 = 144 chunks per partition (n = 18432*rr + f), streamed
as 8 f-sub-tiles of 18 chunks; k/q (and o1/o2) fused on a leading [2]
axis of one DRAM tensor and one SBUF tile.
  * Sub-tile width (2304) % 9 == 0 keeps the stride-9 center-broadcast
    multiply phase-free on every partition (one DVE op per output plane
    covers all 16 channels of a tile).
  * Loads: maximal q-runs constant in (partition, sub-tile, patch) are
    each ONE contiguous DRAM run of a variant image.  One dma_start per
    run covers 16 channels x {k,q} = 32 mutually non-contiguous
    descriptors: the DGE deals descriptors to the 16 SDMA-engine slots
    round-robin from slot 0 and re-aggregates consecutive contiguous
    descriptors, so <16-descriptor (or contiguous-pair) shapes leave
    engines 8-15 idle (HW-measured 86%/39% busy split).
  * Stores: per-channel DRAM is contiguous with offset r*CHF uniform in
    the partition index -> one dma_start per output plane per sub-tile
    (128 descriptors x 4.6 KiB).
  * fp16 end-to-end on device (harness tolerance 2e-2 vs ~7e-4 measured);
    host upcasts to fp32.  Halves HBM read and write traffic vs fp32.

HW-measured dead ends kept out of the design: SBUF->SBUF duplication of
the 3x row overlap between di-patches (engines stream ~20 GB/s
regardless of HBM vs SBUF target, and the extra dependency stage costs
concurrency); dense-center expansion on ACT/GpSimd to unlock the DVE 2x
perf mode (DVE halves but the copies cost more than it saves).

Sharding: pure data-parallel over the 256 (b,c) channels; 32 per core.
"""

import sys

for _p in ("/opt/trn_rl_repo", "/opt/pypackages"):
    if _p not in sys.path:
        sys.path.insert(0, _p)

import numpy as np

import concourse.bass as bass
import concourse.mybir as mybir
import concourse.tile as tile
from concourse.bass import AP
from concourse.bass_utils import run_bass_kernel_spmd
from concourse.vector_clock import ScopedClock

# ---------------------------------------------------------------------------
# Patch: this container's walrus rejects >1 sync-wait on the Tile tail Drain
# ("Too many sync wait commands").  Spill extra waits onto SP NOPs, which
# execute in program order before the all-engine barrier, preserving the
# "all work done before sem clear" semantics.
# ---------------------------------------------------------------------------


def _drain_and_barrier(self, tick_clock, wait_clock):
    nc = self.nc
    drain_inst = nc.sync.drain()
    wait_clock.add_sem_waits(
        drain_inst.ins, ScopedClock({None: tick_clock.global_clock})
    )
    si = drain_inst.ins.sync_info
    if si is not None and len(si.on_wait) > 1:
        waits = list(si.on_wait)
        drain_inst.ins.sync_info = mybir.SyncInfo(
            on_wait=waits[:1], on_update=list(si.on_update)
        )
        for w in waits[1:]:
            nop = nc.sync.nop(nofuse=True)
            nop.ins.sync_info = mybir.SyncInfo(on_wait=[w], on_update=[])

    nc.all_engine_barrier()
    assert self.sems is not None
    popped = nc._tile_sem_poison_stack.pop()
    assert popped is self._sem_poison
    nc.clear_and_free_semaphores(list(self.sems.allocated().values()))
    nc.all_engine_barrier()


tile.TileContext._drain_and_barrier = _drain_and_barrier


def _split_waits(nc, maxw=1):
    """Walrus here allows only `maxw` sync-waits per instruction: move extra
    waits onto same-engine NOPs inserted immediately before the instruction
    (same engine stream => executes before it)."""
    for fn in nc.m.functions:
        for bb in fn.blocks:
            out = []
            for inst in bb.instructions:
                si = getattr(inst, "sync_info", None)
                if si is not None and len(si.on_wait) > maxw:
                    waits = list(si.on_wait)
                    for w in waits[:-maxw]:
                        nop = mybir.InstNoOp(
                            name=nc.get_next_instruction_name(),
                            bass_nofuse=True,
                        )
                        nop.engine = inst.engine
                        nop.sync_info = mybir.SyncInfo(on_wait=[w], on_update=[])
                        nc.register_instruction(nop)
                        out.append(nop)
                    inst.sync_info = mybir.SyncInfo(
                        on_wait=waits[-maxw:], on_update=list(si.on_update)
                    )
                out.append(inst)
            bb.instructions[:] = out

# ---------------------------------------------------------------------------

F16 = mybir.dt.float16

N_CORES = 8
B, C, H, W = 4, 64, 128, 128
BC = B * C                # 256 channels
CPC = BC // N_CORES       # 32 channels per core
NCH = 16                  # channels per tile (x8 partitions = 128)
NG = CPC // NCH           # channel groups per core
HP = H + 2                # padded rows
VAR = HP * W              # one dj-variant: [130, 128]
IMG = 3 * VAR             # three dj-variants per channel
L = H * W
PPCH = 8                  # partitions per channel
CHF = 18432               # elements per partition per channel (144 chunks)
OUT_CH = 9 * L            # 147456 = PPCH * CHF

# Per-group sub-tile chunk widths: each a multiple of 9 (w*128 % 9 == 0)
# so the stride-9 multiply stays phase-free, summing to 144.  Uniform 18
# HW-measured best; end-tapering ([...,9,9]) was within run noise.
SCHED = [
    [18, 18, 18, 18, 18, 18, 18, 18],
    [18, 18, 18, 18, 18, 18, 18, 18],
]
assert all(sum(s) == 144 and all(w * 128 % 9 == 0 for w in s) for s in SCHED)
MAXW = max(w for s in SCHED for w in s)
MAXF = MAXW * 128         # largest sub-tile free width per fused half


def _segments(sched):
    """Per sub-tile: (FREE, f_base, [(rr, f_off, len, src_off), ...]) --
    maximal q-runs inside the sub-tile window not crossing partition
    (144) or patch (128) boundaries.  f_base = chunk offset within CHF."""
    starts = [0]
    for w in sched:
        starts.append(starts[-1] + w)
    tiles = []
    for t, w in enumerate(sched):
        s0 = starts[t]
        bounds = sorted(
            {q for q in range(0, 1153, 144)}
            | {q for q in range(0, 1153, 128)}
            | {144 * r + s for r in range(8) for s in (s0, s0 + w)}
        )
        segs = []
        for qs, qe in zip(bounds[:-1], bounds[1:]):
            rr = qs // 144
            s = qs - 144 * rr
            if not (s0 <= s < s0 + w):
                continue
            p2 = qs // 128
            di, dj = divmod(p2, 3)
            segs.append(
                (rr, (s - s0) * 128, (qe - qs) * 128,
                 dj * VAR + (qs - 128 * p2 + di) * W)
            )
        tiles.append((w * 128, s0 * 128, segs))
    return tiles


_TILES = [_segments(s) for s in SCHED]


def _build_program():
    nc = bass.Bass(trn_type="TRN2")
    # k and q fused on a leading [2] axis (and o1/o2 likewise) so one
    # dma_start covers both: 32-descriptor loads / 256-descriptor stores
    # halve the dma_start count and keep all 16 SDMA engines 2 deep.
    kq = nc.dram_tensor("kq", [2, CPC, 3, HP, W], F16, kind="ExternalInput")
    oo = nc.dram_tensor("oo", [2, CPC, OUT_CH], F16, kind="ExternalOutput")
    IN_X = CPC * IMG          # DRAM stride between k and q planes
    OUT_X = CPC * OUT_CH      # DRAM stride between o1 and o2 planes

    # Three dynamic DMA queues (SP-HWDGE, ACT-HWDGE, Pool-SWDGE); strict
    # round-robin keeps every queue fed (prior HW finding: greedy
    # bin-packing clusters DMAs per queue and the per-engine FIFO then
    # serializes them).
    engines = [nc.sync, nc.scalar, nc.gpsimd]
    eng_i = [0]

    def eng():
        e = engines[eng_i[0] % len(engines)]
        eng_i[0] += 1
        return e

    MF2 = 2 * MAXF  # allocated fused tile free width: k then q (o1 then o2)

    def do_loads(g, free, segs, ti):
        # 32 descriptors (channel-major x {k,q}) per dma_start, mutually
        # non-contiguous in stream order.  Descriptors are dealt to
        # SDMA-engine slots round-robin from slot 0 and consecutive
        # contiguous descriptors re-aggregate into one packet, so
        # 8-descriptor loads pile onto engines 0-7 (HW-measured: 86%
        # busy vs 39% on engines 8-15); 32 descriptors keep all 16
        # engines 2 deep.
        th = ti[:].tensor
        for rr, f_off, seg_len, src_off in segs:
            eng().dma_start(
                AP(th, rr * MF2 + f_off,
                   [[PPCH * MF2, NCH], [free, 2], [1, seg_len]]),
                AP(kq, g * NCH * IMG + src_off,
                   [[IMG, NCH], [IN_X, 2], [1, seg_len]]),
            )

    def do_mul_store(g, free, f_base, ti, ot):
        ith, oth = ti[:].tensor, ot[:].tensor
        ap_o = [[MF2, 128], [9, free // 9], [1, 9]]
        ap_b = [[MF2, 128], [9, free // 9], [0, 9]]
        # o1 = k_full * q_center ; o2 = q_full * k_center.  Each output's
        # store issues right after its own multiply so the second half of
        # the store traffic isn't gated on both muls (shorter drain).
        # (Dense-center rewrites that unlock the DVE 2x perf mode were
        # HW-measured net-neutral: the ACT/GpSimd expansion copies cost
        # more than the DVE multiply saves, and DMA stays the bottleneck.)
        # DRAM per channel is contiguous: partition r = 8*ch + rr maps
        # to offset r*CHF + f_base, uniform across all 128 partitions.
        for x in (0, 1):
            nc.vector.tensor_mul(
                AP(oth, x * free, ap_o),
                AP(ith, x * free, ap_o),
                AP(ith, (1 - x) * free + 4, ap_b),
            )
            eng().dma_start(
                AP(oo, x * OUT_X + g * NCH * OUT_CH + f_base,
                   [[CHF, 128], [1, free]]),
                AP(oth, x * free, [[MF2, 128], [1, free]]),
            )

    with tile.TileContext(nc) as tc:
        with (
            tc.tile_pool(name="tin", bufs=3) as tin,
            tc.tile_pool(name="tout", bufs=3) as tout,
        ):
            # Software pipeline with one-tile lookahead so loads of tile
            # n+1 sit AHEAD of (mul-gated) stores of tile n in each DMA
            # engine's FIFO -> no head-of-line blocking on the loads.
            prev = None
            for g in range(NG):
                for free, f_base, segs in _TILES[g]:
                    ti = tin.tile([128, MF2], F16, tag="ti")
                    do_loads(g, free, segs, ti)
                    if prev is not None:
                        do_mul_store(*prev)
                    ot = tout.tile([128, MF2], F16, tag="ot")
                    prev = (g, free, f_base, ti, ot)
            do_mul_store(*prev)
    _split_waits(nc)
    return nc


_NC_CACHE = []


def _get_nc():
    if not _NC_CACHE:
        _NC_CACHE.append(_build_program())
    return _NC_CACHE[0]


def _variants(x):
    """[B,C,H,W] -> [BC, 3, HP, W] fp16: dj-shifted, row-padded column
    windows of the zero-padded image."""
    xpad = np.pad(
        np.ascontiguousarray(x, dtype=np.float32).reshape(BC, H, W),
        ((0, 0), (1, 1), (1, 1)),
    )
    v = np.stack([xpad[:, :, j : j + W] for j in range(3)], axis=1)
    return np.ascontiguousarray(v.astype(np.float16))


def make_in_maps(key_map, query_map):
    kv = _variants(key_map)
    qv = _variants(query_map)
    return [
        {
            "kq": np.ascontiguousarray(
                np.stack([kv[m * CPC : (m + 1) * CPC],
                          qv[m * CPC : (m + 1) * CPC]])
            ),
        }
        for m in range(N_CORES)
    ]


def assemble(results):
    out1 = np.concatenate([results[m]["oo"][0] for m in range(N_CORES)], axis=0)
    out2 = np.concatenate([results[m]["oo"][1] for m in range(N_CORES)], axis=0)
    return (
        out1.reshape(B, C, L, 9).astype(np.float32),
        out2.reshape(B, C, L, 9).astype(np.float32),
    )


def kernel(key_map, query_map):
    nc = _get_nc()
    in_maps = make_in_maps(key_map, query_map)
    res = run_bass_kernel_spmd(nc, in_maps, core_ids=list(range(N_CORES)))
    return assemble(res.results)


# revision 28
# speedup vs baseline: 1.0472x; 1.0204x over previous
"""Trainium2 Bass kernel for the sparse-attention (local 3x3 unfold) problem.

Math (per batch-channel (b,c), H=W=128, K=3, pad=1):
  ku = unfold(key)  -> [9, L] raw-flat, reinterpreted [L, 9]
  qu = unfold(query)
  out1 = ku * qu[:, 4:5] ; out2 = ku[:, 4:5] * qu   (as [L, 9] views)

The flat per-channel output index n in [0, 9L) decomposes two ways:
  * n = 128*q + j           (chunk q = one (patch p2=q//128, row i2=q%128)
                             slice: 128 contiguous floats of a dj-shifted,
                             row-padded image variant)
  * n = 9*g + e             (group g shares one stride-9 "center" factor)

Device layout: channel ch of a group owns 8 partitions (r = 8*ch + rr)
with CHF = 18432 = 144 chunks per partition (n = 18432*rr + f), streamed
as 8 f-sub-tiles of 18 chunks; k/q (and o1/o2) fused on a leading [2]
axis of one DRAM tensor and one SBUF tile.
  * Sub-tile width (2304) % 9 == 0 keeps the stride-9 center-broadcast
    multiply phase-free on every partition (one DVE op per output plane
    covers all 16 channels of a tile).
  * Loads: maximal q-runs constant in (partition, sub-tile, patch) are
    each ONE contiguous DRAM run of a variant image.  One dma_start per
    run covers 16 channels x {k,q} = 32 mutually non-contiguous
    descriptors: the DGE deals descriptors to the 16 SDMA-engine slots
    round-robin from slot 0 and re-aggregates consecutive contiguous
    descriptors, so <16-descriptor (or contiguous-pair) shapes leave
    engines 8-15 idle (HW-measured 86%/39% busy split).
  * Stores: per-channel DRAM is contiguous with offset r*CHF uniform in
    the partition index -> one dma_start per output plane per sub-tile
    (128 descriptors x 4.6 KiB).
  * fp16 end-to-end on device (harness tolerance 2e-2 vs ~7e-4 measured);
    host upcasts to fp32.  Halves HBM read and write traffic vs fp32.

HW-measured dead ends kept out of the design: SBUF->SBUF duplication of
the 3x row overlap between di-patches (engines stream ~20 GB/s
regardless of HBM vs SBUF target, and the extra dependency stage costs
concurrency); dense-center expansion on ACT/GpSimd to unlock the DVE 2x
perf mode (DVE halves but the copies cost more than it saves).

Sharding: pure data-parallel over the 256 (b,c) channels; 32 per core.
"""

import sys

for _p in ("/opt/trn_rl_repo", "/opt/pypackages"):
    if _p not in sys.path:
        sys.path.insert(0, _p)

import numpy as np

import concourse.bass as bass
import concourse.mybir as mybir
import concourse.tile as tile
from concourse.bass import AP
from concourse.bass_utils import run_bass_kernel_spmd
from concourse.vector_clock import ScopedClock

# ---------------------------------------------------------------------------
# Patch: this container's walrus rejects >1 sync-wait on the Tile tail Drain
# ("Too many sync wait commands").  Spill extra waits onto SP NOPs, which
# execute in program order before the all-engine barrier, preserving the
# "all work done before sem clear" semantics.
# ---------------------------------------------------------------------------


def _drain_and_barrier(self, tick_clock, wait_clock):
    nc = self.nc
    drain_inst = nc.sync.drain()
    wait_clock.add_sem_waits(
        drain_inst.ins, ScopedClock({None: tick_clock.global_clock})
    )
    si = drain_inst.ins.sync_info
    if si is not None and len(si.on_wait) > 1:
        waits = list(si.on_wait)
        drain_inst.ins.sync_info = mybir.SyncInfo(
            on_wait=waits[:1], on_update=list(si.on_update)
        )
        for w in waits[1:]:
            nop = nc.sync.nop(nofuse=True)
            nop.ins.sync_info = mybir.SyncInfo(on_wait=[w], on_update=[])

    nc.all_engine_barrier()
    assert self.sems is not None
    popped = nc._tile_sem_poison_stack.pop()
    assert popped is self._sem_poison
    nc.clear_and_free_semaphores(list(self.sems.allocated().values()))
    nc.all_engine_barrier()


tile.TileContext._drain_and_barrier = _drain_and_barrier


def _split_waits(nc, maxw=1):
    """Walrus here allows only `maxw` sync-waits per instruction: move extra
    waits onto same-engine NOPs inserted immediately before the instruction
    (same engine stream => executes before it)."""
    for fn in nc.m.functions:
        for bb in fn.blocks:
            out = []
            for inst in bb.instructions:
                si = getattr(inst, "sync_info", None)
                if si is not None and len(si.on_wait) > maxw:
                    waits = list(si.on_wait)
                    for w in waits[:-maxw]:
                        nop = mybir.InstNoOp(
                            name=nc.get_next_instruction_name(),
                            bass_nofuse=True,
                        )
                        nop.engine = inst.engine
                        nop.sync_info = mybir.SyncInfo(on_wait=[w], on_update=[])
                        nc.register_instruction(nop)
                        out.append(nop)
                    inst.sync_info = mybir.SyncInfo(
                        on_wait=waits[-maxw:], on_update=list(si.on_update)
                    )
                out.append(inst)
            bb.instructions[:] = out

# ---------------------------------------------------------------------------

F16 = mybir.dt.float16

N_CORES = 8
B, C, H, W = 4, 64, 128, 128
BC = B * C                # 256 channels
CPC = BC // N_CORES       # 32 channels per core
NCH = 16                  # channels per tile (x8 partitions = 128)
NG = CPC // NCH           # channel groups per core
HP = H + 2                # padded rows
VAR = HP * W              # one dj-variant: [130, 128]
IMG = 3 * VAR             # three dj-variants per channel
L = H * W
PPCH = 8                  # partitions per channel
CHF = 18432               # elements per partition per channel (144 chunks)
OUT_CH = 9 * L            # 147456 = PPCH * CHF

# Per-group sub-tile chunk widths: each a multiple of 9 (w*128 % 9 == 0)
# so the stride-9 multiply stays phase-free, summing to 144.  Uniform 18
# HW-measured best; end-tapering ([...,9,9]) was within run noise.
SCHED = [
    [18, 18, 18, 18, 18, 18, 18, 18],
    [18, 18, 18, 18, 18, 18, 18, 18],
]
assert all(sum(s) == 144 and all(w * 128 % 9 == 0 for w in s) for s in SCHED)
MAXW = max(w for s in SCHED for w in s)
MAXF = MAXW * 128         # largest sub-tile free width per fused half


def _segments(sched):
    """Per sub-tile: (FREE, f_base, [(rr, f_off, len, src_off), ...]) --
    maximal q-runs inside the sub-tile window not crossing partition
    (144) or patch (128) boundaries.  f_base = chunk offset within CHF."""
    starts = [0]
    for w in sched:
        starts.append(starts[-1] + w)
    tiles = []
    for t, w in enumerate(sched):
        s0 = starts[t]
        bounds = sorted(
            {q for q in range(0, 1153, 144)}
            | {q for q in range(0, 1153, 128)}
            | {144 * r + s for r in range(8) for s in (s0, s0 + w)}
        )
        segs = []
        for qs, qe in zip(bounds[:-1], bounds[1:]):
            rr = qs // 144
            s = qs - 144 * rr
            if not (s0 <= s < s0 + w):
                continue
            p2 = qs // 128
            di, dj = divmod(p2, 3)
            segs.append(
                (rr, (s - s0) * 128, (qe - qs) * 128,
                 dj * VAR + (qs - 128 * p2 + di) * W)
            )
        tiles.append((w * 128, s0 * 128, segs))
    return tiles


_TILES = [_segments(s) for s in SCHED]


def _build_program():
    nc = bass.Bass(trn_type="TRN2")
    # k and q fused on a leading [2] axis (and o1/o2 likewise) so one
    # dma_start covers both: 32-descriptor loads / 256-descriptor stores
    # halve the dma_start count and keep all 16 SDMA engines 2 deep.
    kq = nc.dram_tensor("kq", [2, CPC, 3, HP, W], F16, kind="ExternalInput")
    oo = nc.dram_tensor("oo", [2, CPC, OUT_CH], F16, kind="ExternalOutput")
    IN_X = CPC * IMG          # DRAM stride between k and q planes
    OUT_X = CPC * OUT_CH      # DRAM stride between o1 and o2 planes

    # Three dynamic DMA queues (SP-HWDGE, ACT-HWDGE, Pool-SWDGE); strict
    # round-robin keeps every queue fed (prior HW finding: greedy
    # bin-packing clusters DMAs per queue and the per-engine FIFO then
    # serializes them).
    engines = [nc.sync, nc.scalar, nc.gpsimd]
    eng_i = [0]

    def eng():
        e = engines[eng_i[0] % len(engines)]
        eng_i[0] += 1
        return e

    MF2 = 2 * MAXF  # allocated fused tile free width: k then q (o1 then o2)

    def do_loads(g, free, segs, ti):
        # 32 descriptors (channel-major x {k,q}) per dma_start, mutually
        # non-contiguous in stream order.  Descriptors are dealt to
        # SDMA-engine slots round-robin from slot 0 and consecutive
        # contiguous descriptors re-aggregate into one packet, so
        # 8-descriptor loads pile onto engines 0-7 (HW-measured: 86%
        # busy vs 39% on engines 8-15); 32 descriptors keep all 16
        # engines 2 deep.
        th = ti[:].tensor
        for rr, f_off, seg_len, src_off in segs:
            eng().dma_start(
                AP(th, rr * MF2 + f_off,
                   [[PPCH * MF2, NCH], [free, 2], [1, seg_len]]),
                AP(kq, g * NCH * IMG + src_off,
                   [[IMG, NCH], [IN_X, 2], [1, seg_len]]),
            )

    def do_cen(free, ti, cen):
        # Expand ONLY the q-centers (o1's broadcast operand) into a dense
        # tile on the otherwise-idle ACT engine.  The o1 multiply then has
        # all operands packed step-1 fp16 -> DVE 2x perf mode (HW-measured:
        # DVE busy halves); o2 keeps the broadcast form on DVE.  Expanding
        # BOTH planes on ACT was HW-measured net-neutral (ACT ~99us became
        # the pacer); this hybrid balances ACT ~72us / DVE ~58us, both
        # under the ~120us DMA floor.
        nc.scalar.activation(
            AP(cen[:].tensor, 0, [[MAXF, 128], [9, free // 9], [1, 9]]),
            AP(ti[:].tensor, free + 4, [[MF2, 128], [9, free // 9], [0, 9]]),
            mybir.ActivationFunctionType.Copy,
        )

    def do_mul_store(g, free, f_base, ti, cen, ot):
        ith, cenh, oth = ti[:].tensor, cen[:].tensor, ot[:].tensor
        ap_d = [[MF2, 128], [1, free]]
        ap_o = [[MF2, 128], [9, free // 9], [1, 9]]
        ap_b = [[MF2, 128], [9, free // 9], [0, 9]]
        # o1 = k_full * q_center (dense, 2x) ; o2 = q_full * k_center
        # (broadcast, 1x).  Each output's store issues right after its own
        # multiply so the second store isn't gated on both muls.
        # DRAM per channel is contiguous: partition r = 8*ch + rr maps
        # to offset r*CHF + f_base, uniform across all 128 partitions.
        nc.vector.tensor_mul(
            AP(oth, 0, ap_d),
            AP(ith, 0, ap_d),
            AP(cenh, 0, [[MAXF, 128], [1, free]]),
        )
        eng().dma_start(
            AP(oo, g * NCH * OUT_CH + f_base, [[CHF, 128], [1, free]]),
            AP(oth, 0, [[MF2, 128], [1, free]]),
        )
        nc.vector.tensor_mul(
            AP(oth, free, ap_o),
            AP(ith, free, ap_o),
            AP(ith, 4, ap_b),
        )
        eng().dma_start(
            AP(oo, OUT_X + g * NCH * OUT_CH + f_base, [[CHF, 128], [1, free]]),
            AP(oth, free, [[MF2, 128], [1, free]]),
        )

    with tile.TileContext(nc) as tc:
        with (
            tc.tile_pool(name="tin", bufs=4) as tin,
            tc.tile_pool(name="tcen", bufs=3) as tcen,
            tc.tile_pool(name="tout", bufs=3) as tout,
        ):
            # Two-stage lookahead: loads(n) | q-center copy(n-1) |
            # muls+stores(n-2).  DMA-queue FIFOs see all loads ahead of
            # the mul-gated stores, and the ACT center-copy (which waits
            # on loads n-1) never head-of-line-blocks the dma_starts ACT
            # issues for tile n.
            stage = []
            for g in range(NG):
                for free, f_base, segs in _TILES[g]:
                    ti = tin.tile([128, MF2], F16, tag="ti")
                    do_loads(g, free, segs, ti)
                    cen = tcen.tile([128, MAXF], F16, tag="cen")
                    ot = tout.tile([128, MF2], F16, tag="ot")
                    stage.append((g, free, f_base, ti, cen, ot))
                    if len(stage) >= 2:
                        do_cen(stage[-2][1], stage[-2][3], stage[-2][4])
                    if len(stage) >= 3:
                        do_mul_store(*stage[-3])
            do_cen(stage[-1][1], stage[-1][3], stage[-1][4])
            do_mul_store(*stage[-2])
            do_mul_store(*stage[-1])
    _split_waits(nc)
    return nc


_NC_CACHE = []


def _get_nc():
    if not _NC_CACHE:
        _NC_CACHE.append(_build_program())
    return _NC_CACHE[0]


def _variants(x):
    """[B,C,H,W] -> [BC, 3, HP, W] fp16: dj-shifted, row-padded column
    windows of the zero-padded image."""
    xpad = np.pad(
        np.ascontiguousarray(x, dtype=np.float32).reshape(BC, H, W),
        ((0, 0), (1, 1), (1, 1)),
    )
    v = np.stack([xpad[:, :, j : j + W] for j in range(3)], axis=1)
    return np.ascontiguousarray(v.astype(np.float16))


def make_in_maps(key_map, query_map):
    kv = _variants(key_map)
    qv = _variants(query_map)
    return [
        {
            "kq": np.ascontiguousarray(
                np.stack([kv[m * CPC : (m + 1) * CPC],
                          qv[m * CPC : (m + 1) * CPC]])
            ),
        }
        for m in range(N_CORES)
    ]


def assemble(results):
    out1 = np.concatenate([results[m]["oo"][0] for m in range(N_CORES)], axis=0)
    out2 = np.concatenate([results[m]["oo"][1] for m in range(N_CORES)], axis=0)
    return (
        out1.reshape(B, C, L, 9).astype(np.float32),
        out2.reshape(B, C, L, 9).astype(np.float32),
    )


def kernel(key_map, query_map):
    nc = _get_nc()
    in_maps = make_in_maps(key_map, query_map)
    res = run_bass_kernel_spmd(nc, in_maps, core_ids=list(range(N_CORES)))
    return assemble(res.results)
